# revision 12
# baseline (speedup 1.0000x reference)
"""Trainium2 Bass kernel for nn_DecoderHead (B=2, T=2048, D=1024, H=16, DH=64).

y = x + softmax_causal((x @ Wq.T) split to heads @ k^T / sqrt(D)) @ v

Sharding: 8 cores = 2 (batch) x 4 (head groups of 4 heads). Each core computes
its batch's q-projection for its 256 output features, causal attention for its
4 heads, adds the residual slice, and writes a [T, 256] slice; the host
concatenates slices.

All matmul operands are fp8e4 (e4m3): x/Wq for the q-projection, q/k for the
scores, exp(s)/v for the PV product (PSUM accumulates fp32). On TRN2 the PE
streams 1 moving row/cycle regardless of dtype, so fp8 buys DMA volume, SBUF
footprint and switching power (less HAM duty-cycle throttling), not rate.
exp(s) runs on the ACT engine straight out of PSUM into fp8 et tiles. The
causal diagonal is rect-trimmed: QK/exp/PV only touch the valid column range
of each key-block pair, and a single fused [128,2,256] mask multiply
(zeros + triangle) on the Pool engine zeroes the rest. Epilogue (PE transpose,
1/denom, residual add) is bf16 end-to-end; y returns bf16, upcast on host.
"""

import os
from collections import deque

import numpy as np
import ml_dtypes

import concourse.bass as bass
import concourse.mybir as mybir
import concourse.tile as tile
from concourse import bacc
from concourse.alu_op_type import AluOpType
from concourse.bass_utils import run_bass_kernel_spmd

# Problem shape (hardcoded per the harness contract).
B, T, D, H = 2, 2048, 1024, 16
DH = D // H          # 64
N_CORES = 8
HPC = H // (N_CORES // B)   # heads per core = 4
EPC = HPC * DH       # output features per core = 256
P = 128              # SBUF partitions
TQ = 512             # query-tile width
NTQ = T // TQ        # 4
NTKB = T // P        # 16 key blocks of 128
DT = D // P          # 8 contraction passes for qproj
EG = 2               # head-pair groups (2 heads per 128 partitions)
VP = DH + 1          # 65 = v columns + denominator ones-row
SCALE = 1.0 / np.sqrt(np.float32(D))   # 1/32

F32 = mybir.dt.float32
BF16 = mybir.dt.bfloat16
FP8 = mybir.dt.float8e4

VARIANT = os.environ.get("DH_VARIANT", "fp8")


def build_nc(variant: str = VARIANT, repeat: int = 1):
    """Build the per-core SPMD Bass program. `repeat` wraps the body in a
    hardware loop (timing only)."""
    nc = bacc.Bacc(
        "TRN2", target_bir_lowering=False, debug=False, num_devices=N_CORES
    )

    xT8 = nc.dram_tensor("xT8", [P, DT, T], FP8, kind="ExternalInput").ap()
    wq8 = nc.dram_tensor("wq8", [P, DT, EPC], FP8, kind="ExternalInput").ap()
    kT8 = nc.dram_tensor("kT8", [P, EG, NTKB, P], FP8,
                         kind="ExternalInput").ap()
    vO8 = nc.dram_tensor("vO8", [P, NTKB, HPC, VP], FP8,
                         kind="ExternalInput").ap()
    xr = nc.dram_tensor("xr", [P, NTKB, EPC], BF16, kind="ExternalInput").ap()
    dm8 = nc.dram_tensor("dm8", [P, 2, 2 * P], FP8, kind="ExternalInput").ap()
    ident = nc.dram_tensor("ident", [P, P], BF16, kind="ExternalInput").ap()
    y = nc.dram_tensor("y", [T, EPC], BF16, kind="ExternalOutput").ap()

    with tile.TileContext(nc) as tc:
        with (
            tc.tile_pool(name="const", bufs=1) as cpool,
            tc.tile_pool(name="xq", bufs=1) as xqpool,
            tc.tile_pool(name="work", bufs=8) as wpool,
            tc.tile_pool(name="epi", bufs=2) as epool,
            tc.tile_pool(name="ps_s", bufs=3, space="PSUM") as ps_s,
            tc.tile_pool(name="ps_o", bufs=2, space="PSUM") as ps_o,
        ):
            def body(_iv=None):
                # ---- tiles -------------------------------------------------
                id_sb = cpool.tile([P, P], BF16, name="id_sb", tag="id_sb")
                dm_sb = cpool.tile([P, 2, 2 * P], FP8, name="dm_sb",
                                   tag="dm_sb")
                wq_sb = xqpool.tile([P, DT, EPC], FP8, name="wq_sb",
                                    tag="wq_sb")
                xT_sb = xqpool.tile([P, DT, T], FP8, name="xT_sb",
                                    tag="xT_sb")
                kT_sb = cpool.tile([P, EG, NTKB, P], FP8, name="kT_sb",
                                   tag="kT_sb")
                vO_sb = cpool.tile([P, NTKB, HPC, VP], FP8,
                                   name="vO_sb", tag="vO_sb")
                xr_sb = cpool.tile([P, NTKB, EPC], BF16, name="xr_sb",
                                   tag="xr_sb")
                qT_sb = xqpool.tile([P, EG, T], FP8, name="qT_sb",
                                    tag="qT_sb")

                # ---- warm-up first: no DMA dependency (memset stationary),
                # primes the ACT exp table and opens the HAM clock-gate while
                # the first DMAs stream in.
                warm_w = wpool.tile([P, TQ], BF16, name="warm_w", tag="warm")
                warm_et = wpool.tile([P, P], BF16, name="warm_et", tag="warm")
                nc.vector.memset(warm_w[:], 0)
                psw = ps_o.tile([P, TQ], F32, name="psw", tag="o")
                for w in range(12):
                    nc.tensor.matmul(
                        psw[:], warm_w[:, 0:P], warm_w[:], start=True,
                        stop=True,
                    )
                nc.scalar.activation(
                    warm_et[:], psw[:, 0:P],
                    mybir.ActivationFunctionType.Exp, scale=0.01,
                )

                # ---- stage-0 loads ----------------------------------------
                nc.sync.dma_start(wq_sb[:], wq8[:])

                def load_stage(c, split_x=False):
                    """Inputs first needed by tq-tile c."""
                    sl = bass.ts(c, TQ)
                    if split_x:
                        # per-contraction-pass pieces: qproj pass dt can start
                        # as soon as its own slice lands
                        for dt_i in range(DT):
                            nc.sync.dma_start(xT_sb[:, dt_i, sl],
                                              xT8[:, dt_i, sl])
                    else:
                        nc.sync.dma_start(xT_sb[:, :, sl], xT8[:, :, sl])
                    nc.sync.dma_start(kT_sb[:, :, 4 * c:4 * (c + 1)],
                                      kT8[:, :, 4 * c:4 * (c + 1)])
                    nc.sync.dma_start(vO_sb[:, 4 * c:4 * (c + 1)],
                                      vO8[:, 4 * c:4 * (c + 1)])
                    nc.sync.dma_start(xr_sb[:, 4 * c:4 * (c + 1)],
                                      xr[:, 4 * c:4 * (c + 1)])

                load_stage(0, split_x=True)
                nc.sync.dma_start(id_sb[:], ident[:])
                nc.sync.dma_start(dm_sb[:], dm8[:])

                pending = deque()

                def epilogue_start(h, tqt, pso_t):
                    oT = epool.tile([VP, TQ], BF16, name="oT", tag="oT",
                                    bufs=4)
                    nc.vector.tensor_copy(oT[:], pso_t[:])
                    return (h, tqt, oT)

                def epilogue(state):
                    h, tqt, oT = state
                    ysb = epool.tile([P, 4, DH], BF16, name="ysb", tag="ysb")
                    # VP+1 columns keep each j-slice 4-byte aligned in PSUM
                    pst = ps_o.tile([P, 4, VP + 1], BF16, name="pst", tag="o")
                    for j in range(4):
                        nc.tensor.transpose(
                            pst[:, j, 0:VP],
                            oT[:, j * P:(j + 1) * P],
                            id_sb[0:VP, 0:VP],
                        )
                    rc = epool.tile([P, 4], F32, name="rc", tag="rc", bufs=4)
                    nc.vector.reciprocal(rc[:], pst[:, :, DH])
                    for j in range(4):
                        nc.vector.scalar_tensor_tensor(
                            ysb[:, j, :],
                            pst[:, j, 0:DH],
                            rc[:, j:j + 1],
                            xr_sb[:, 4 * tqt + j, h * DH:(h + 1) * DH],
                            AluOpType.mult,
                            AluOpType.add,
                        )
                    ydst = y[tqt * TQ:(tqt + 1) * TQ, h * DH:(h + 1) * DH]
                    nc.sync.dma_start(
                        ydst.rearrange("(j p) c -> p j c", p=P), ysb[:]
                    )

                def attention(h, tqt):
                    g = h >> 1
                    rb = DH * (h & 1)
                    npair = 2 * (tqt + 1)
                    tq0 = tqt * TQ
                    pso = ps_o.tile([VP, TQ], F32, name="pso", tag="o")

                    def emit_pv(et, mp, offs, last=False):
                        for u in range(2):
                            nc.tensor.matmul(
                                pso[:, offs[u]:TQ],
                                vO_sb[:, 2 * mp + u, h, 0:VP],
                                et[:, u, offs[u]:TQ],
                                start=(mp == 0 and u == 0),
                                stop=(last and u == 1),
                                skip_group_check=True,
                            )

                    prev = None
                    for mp in range(npair):
                        # per-block valid-column offsets within the tq tile
                        # (0 off-diagonal; 128*m - tq0 on the diagonal band)
                        offs = [max(0, P * (2 * mp + u) - tq0) for u in (0, 1)]
                        diag = offs[1] > 0
                        pssc = ps_s.tile([P, 2, TQ], F32, name="pssc", tag="s")
                        for u in range(2):
                            m = 2 * mp + u
                            nc.tensor.matmul(
                                pssc[:, u, offs[u]:TQ],
                                kT_sb[rb:rb + DH, g, m, :],
                                qT_sb[rb:rb + DH, g,
                                      tq0 + offs[u]:tq0 + TQ],
                                start=True,
                                stop=True,
                            )
                        et = wpool.tile([P, 2, TQ], FP8, name="et", tag="et")
                        if diag:
                            for u in range(2):
                                nc.scalar.activation(
                                    et[:, u, offs[u]:TQ],
                                    pssc[:, u, offs[u]:TQ],
                                    mybir.ActivationFunctionType.Exp,
                                    scale=float(SCALE),
                                )
                                # triangle at the block's diagonal corner
                                nc.vector.tensor_mul(
                                    et[:, u, offs[u]:offs[u] + P],
                                    et[:, u, offs[u]:offs[u] + P],
                                    dm_sb[:, 0, 0:P],
                                )
                        else:
                            nc.scalar.activation(
                                et[:, :, 0:TQ], pssc[:, :, 0:TQ],
                                mybir.ActivationFunctionType.Exp,
                                scale=float(SCALE),
                            )
                        if prev is not None:
                            emit_pv(*prev)
                        prev = (et, mp, offs)
                        if pending and mp >= 1:
                            epilogue(pending.popleft())
                    emit_pv(*prev, last=True)
                    pending.append(epilogue_start(h, tqt, pso))

                def qproj(tqc):
                    sl = bass.ts(tqc, TQ)
                    for g in range(EG):
                        psq = ps_s.tile([P, TQ], F32, name="psq", tag="s")
                        for dt_i in range(DT):
                            nc.tensor.matmul(
                                psq[:],
                                wq_sb[:, dt_i, P * g:P * (g + 1)],
                                xT_sb[:, dt_i, sl],
                                start=(dt_i == 0),
                                stop=(dt_i == DT - 1),
                            )
                        nc.vector.tensor_copy(qT_sb[:, g, sl], psq[:])

                # ---- main schedule: qproj for tile t, then attention over
                # its heads while stage t+1 streams in.
                qproj(0)
                for tqt in range(NTQ):
                    if tqt + 1 < NTQ:
                        load_stage(tqt + 1)
                    for h in range(HPC):
                        attention(h, tqt)
                        if h == 0 and tqt + 1 < NTQ:
                            qproj(tqt + 1)
                while pending:
                    epilogue(pending.popleft())

            if repeat == 1:
                body()
            else:
                tc.For_i_unrolled(0, repeat, 1, body, max_unroll=1)

    nc.compile()
    return nc


def _f8(a: np.ndarray) -> np.ndarray:
    return np.ascontiguousarray(a, dtype=np.float32).astype(
        ml_dtypes.float8_e4m3
    )


def _b16(a: np.ndarray) -> np.ndarray:
    return np.ascontiguousarray(a, dtype=np.float32).astype(ml_dtypes.bfloat16)


def prep_in_maps(x, k, v, Wq, variant: str = VARIANT):
    """Build the 8 per-core input maps from full inputs (host-side numpy)."""
    x = np.asarray(x, dtype=np.float32)
    k = np.asarray(k, dtype=np.float32)
    v = np.asarray(v, dtype=np.float32)
    Wq = np.asarray(Wq, dtype=np.float32)

    # dm8: fused diagonal mask for a key-block pair at its diagonal corner.
    i_ = np.arange(P)[:, None]
    j_ = np.arange(P)[None, :]
    tri = (i_ <= j_).astype(np.float32)
    dm = np.empty((P, 2, 2 * P), dtype=np.float32)
    dm[:, 0, 0:P] = tri
    dm[:, 0, P:] = 1.0
    dm[:, 1, 0:P] = 0.0
    dm[:, 1, P:] = tri
    ident = np.eye(P, dtype=np.float32)

    in_maps = []
    for c in range(N_CORES):
        b = c // (N_CORES // B)
        grp = c % (N_CORES // B)
        heads = slice(HPC * grp, HPC * (grp + 1))
        cols = slice(EPC * grp, EPC * (grp + 1))

        # xT8[p, dt, t] = x[b, t, 128*dt + p]
        xT8 = x[b].T.reshape(DT, P, T).transpose(1, 0, 2)
        # wq8[p, dt, f] = Wq[cols][f, 128*dt + p]
        wq8 = Wq[cols, :].T.reshape(DT, P, EPC).transpose(1, 0, 2)
        # kT8[64*(h&1)+dh, h>>1, m, tk] = k[b, head h, 128m+tk, dh]
        kT8 = np.zeros((P, EG, NTKB, P), dtype=np.float32)
        for hl in range(HPC):
            g, hb = hl // 2, hl % 2
            kh = k[b, HPC * grp + hl].reshape(NTKB, P, DH)
            kT8[DH * hb:DH * hb + DH, g, :, :] = kh.transpose(2, 0, 1)
        # vO8[p, m, h, dh'] = v[b, head h, 128m+p, dh'], ones at dh'=64
        vO8 = np.ones((P, NTKB, HPC, VP), dtype=np.float32)
        vh = v[b, heads].reshape(HPC, NTKB, P, DH)
        vO8[:, :, :, 0:DH] = vh.transpose(2, 1, 0, 3)
        xres = np.ascontiguousarray(
            x[b][:, cols].reshape(NTKB, P, EPC).transpose(1, 0, 2)
        )
        in_maps.append({
            "xT8": _f8(xT8),
            "wq8": _f8(wq8),
            "kT8": _f8(kT8),
            "vO8": _f8(vO8),
            "xr": _b16(xres),
            "dm8": _f8(dm),
            "ident": _b16(ident),
        })
    return in_maps


def gather_output(results):
    """Assemble full [B, T, D] output from 8 per-core [T, EPC] slices."""
    y = np.empty((B, T, D), dtype=np.float32)
    for c in range(N_CORES):
        b = c // (N_CORES // B)
        grp = c % (N_CORES // B)
        y[b, :, EPC * grp:EPC * (grp + 1)] = np.asarray(
            results[c]["y"], dtype=np.float32
        )
    return y


_NC_CACHE = {}


def kernel(x, k, v, Wq):
    key = (VARIANT, 1)
    if key not in _NC_CACHE:
        _NC_CACHE[key] = build_nc(VARIANT, repeat=1)
    nc = _NC_CACHE[key]
    in_maps = prep_in_maps(x, k, v, Wq, VARIANT)
    res = run_bass_kernel_spmd(nc, in_maps, core_ids=list(range(N_CORES)))
    return gather_output(res.results)


# revision 14
# speedup vs baseline: 1.0234x; 1.0234x over previous
"""Trainium2 Bass kernel for nn_DecoderHead (B=2, T=2048, D=1024, H=16, DH=64).

y = x + softmax_causal((x @ Wq.T) split to heads @ k^T / sqrt(D)) @ v

Sharding: 8 cores = 2 (batch) x 4 (head groups of 4 heads). Each core computes
its batch's q-projection for its 256 output features, causal attention for its
4 heads, adds the residual slice, and writes a [T, 256] slice; the host
concatenates slices.

All matmul operands are fp8e4 (e4m3): x/Wq for the q-projection, q/k for the
scores, exp(s)/v for the PV product (PSUM accumulates fp32). On TRN2 the PE
streams 1 moving row/cycle regardless of dtype, so fp8 buys DMA volume, SBUF
footprint and switching power (less HAM duty-cycle throttling), not rate.
exp(s) runs on the ACT engine straight out of PSUM into fp8 et tiles. The
causal diagonal is rect-trimmed: QK/exp/PV only touch the valid column range
of each key-block pair, and a single fused [128,2,256] mask multiply
(zeros + triangle) on the Pool engine zeroes the rest. Epilogue (PE transpose,
1/denom, residual add) is bf16 end-to-end; y returns bf16, upcast on host.
"""

import os
from collections import deque

import numpy as np
import ml_dtypes

import concourse.bass as bass
import concourse.mybir as mybir
import concourse.tile as tile
from concourse import bacc
from concourse.alu_op_type import AluOpType
from concourse.bass_utils import run_bass_kernel_spmd

# Problem shape (hardcoded per the harness contract).
B, T, D, H = 2, 2048, 1024, 16
DH = D // H          # 64
N_CORES = 8
HPC = H // (N_CORES // B)   # heads per core = 4
EPC = HPC * DH       # output features per core = 256
P = 128              # SBUF partitions
TQ = 512             # query-tile width
NTQ = T // TQ        # 4
NTKB = T // P        # 16 key blocks of 128
DT = D // P          # 8 contraction passes for qproj
EG = 2               # head-pair groups (2 heads per 128 partitions)
VP = DH + 1          # 65 = v columns + denominator ones-row
SCALE = 1.0 / np.sqrt(np.float32(D))   # 1/32

F32 = mybir.dt.float32
BF16 = mybir.dt.bfloat16
FP8 = mybir.dt.float8e4

VARIANT = os.environ.get("DH_VARIANT", "fp8")


def build_nc(variant: str = VARIANT, repeat: int = 1):
    """Build the per-core SPMD Bass program. `repeat` wraps the body in a
    hardware loop (timing only)."""
    nc = bacc.Bacc(
        "TRN2", target_bir_lowering=False, debug=False, num_devices=N_CORES
    )

    xT8 = nc.dram_tensor("xT8", [P, DT, T], FP8, kind="ExternalInput").ap()
    wq8 = nc.dram_tensor("wq8", [P, DT, EPC], FP8, kind="ExternalInput").ap()
    kT8 = nc.dram_tensor("kT8", [P, EG, NTKB, P], FP8,
                         kind="ExternalInput").ap()
    vO8 = nc.dram_tensor("vO8", [P, NTKB, HPC, VP], FP8,
                         kind="ExternalInput").ap()
    xr = nc.dram_tensor("xr", [P, NTKB, EPC], BF16, kind="ExternalInput").ap()
    dm8 = nc.dram_tensor("dm8", [P, 2, 2 * P], FP8, kind="ExternalInput").ap()
    ident = nc.dram_tensor("ident", [P, P], BF16, kind="ExternalInput").ap()
    y = nc.dram_tensor("y", [T, EPC], BF16, kind="ExternalOutput").ap()

    with tile.TileContext(nc) as tc:
        with (
            tc.tile_pool(name="const", bufs=1) as cpool,
            tc.tile_pool(name="xq", bufs=1) as xqpool,
            tc.tile_pool(name="work", bufs=8) as wpool,
            tc.tile_pool(name="epi", bufs=2) as epool,
            tc.tile_pool(name="ps_s", bufs=3, space="PSUM") as ps_s,
            tc.tile_pool(name="ps_o", bufs=2, space="PSUM") as ps_o,
        ):
            def body(_iv=None):
                # ---- tiles -------------------------------------------------
                id_sb = cpool.tile([P, P], BF16, name="id_sb", tag="id_sb")
                dm_sb = cpool.tile([P, 2, 2 * P], FP8, name="dm_sb",
                                   tag="dm_sb")
                wq_sb = xqpool.tile([P, DT, EPC], FP8, name="wq_sb",
                                    tag="wq_sb")
                xT_sb = xqpool.tile([P, DT, T], FP8, name="xT_sb",
                                    tag="xT_sb")
                kT_sb = cpool.tile([P, EG, NTKB, P], FP8, name="kT_sb",
                                   tag="kT_sb")
                vO_sb = cpool.tile([P, NTKB, HPC, VP], FP8,
                                   name="vO_sb", tag="vO_sb")
                xr_sb = cpool.tile([P, NTKB, EPC], BF16, name="xr_sb",
                                   tag="xr_sb")
                qT_sb = xqpool.tile([P, EG, T], FP8, name="qT_sb",
                                    tag="qT_sb")

                # ---- warm-up first: no DMA dependency (memset stationary),
                # primes the ACT exp table and opens the HAM clock-gate while
                # the first DMAs stream in.
                warm_w = wpool.tile([P, TQ], BF16, name="warm_w", tag="warm")
                warm_et = wpool.tile([P, P], BF16, name="warm_et", tag="warm")
                nc.vector.memset(warm_w[:], 0)
                psw = ps_o.tile([P, TQ], F32, name="psw", tag="o")
                for w in range(12):
                    nc.tensor.matmul(
                        psw[:], warm_w[:, 0:P], warm_w[:], start=True,
                        stop=True,
                    )
                nc.scalar.activation(
                    warm_et[:], psw[:, 0:P],
                    mybir.ActivationFunctionType.Exp, scale=0.01,
                )
                # Zero the score PSUM buffers once: diagonal-pair exp reads a
                # stale sliver of PSUM (masked to zero afterwards) which must
                # be finite even on the very first use.
                for _z in range(3):
                    psz = ps_s.tile([P, 2, TQ], F32, name="psz", tag="s")
                    nc.vector.memset(psz[:], 0)

                # ---- stage-0 loads ----------------------------------------
                nc.sync.dma_start(wq_sb[:], wq8[:])

                def load_stage(c, split_x=False):
                    """Inputs first needed by tq-tile c."""
                    sl = bass.ts(c, TQ)
                    if split_x:
                        # per-contraction-pass pieces: qproj pass dt can start
                        # as soon as its own slice lands
                        for dt_i in range(DT):
                            nc.sync.dma_start(xT_sb[:, dt_i, sl],
                                              xT8[:, dt_i, sl])
                    else:
                        nc.sync.dma_start(xT_sb[:, :, sl], xT8[:, :, sl])
                    nc.sync.dma_start(kT_sb[:, :, 4 * c:4 * (c + 1)],
                                      kT8[:, :, 4 * c:4 * (c + 1)])
                    nc.sync.dma_start(vO_sb[:, 4 * c:4 * (c + 1)],
                                      vO8[:, 4 * c:4 * (c + 1)])
                    nc.sync.dma_start(xr_sb[:, 4 * c:4 * (c + 1)],
                                      xr[:, 4 * c:4 * (c + 1)])

                load_stage(0, split_x=True)
                nc.sync.dma_start(id_sb[:], ident[:])
                nc.sync.dma_start(dm_sb[:], dm8[:])

                pending = deque()

                def epilogue_start(h, tqt, pso_t):
                    oT = epool.tile([VP, TQ], BF16, name="oT", tag="oT",
                                    bufs=4)
                    nc.vector.tensor_copy(oT[:], pso_t[:])
                    return (h, tqt, oT)

                def epilogue(state):
                    h, tqt, oT = state
                    ysb = epool.tile([P, 4, DH], BF16, name="ysb", tag="ysb")
                    # VP+1 columns keep each j-slice 4-byte aligned in PSUM
                    pst = ps_o.tile([P, 4, VP + 1], BF16, name="pst", tag="o")
                    for j in range(4):
                        nc.tensor.transpose(
                            pst[:, j, 0:VP],
                            oT[:, j * P:(j + 1) * P],
                            id_sb[0:VP, 0:VP],
                        )
                    rc = epool.tile([P, 4], F32, name="rc", tag="rc", bufs=4)
                    nc.vector.reciprocal(rc[:], pst[:, :, DH])
                    for j in range(4):
                        nc.vector.scalar_tensor_tensor(
                            ysb[:, j, :],
                            pst[:, j, 0:DH],
                            rc[:, j:j + 1],
                            xr_sb[:, 4 * tqt + j, h * DH:(h + 1) * DH],
                            AluOpType.mult,
                            AluOpType.add,
                        )
                    ydst = y[tqt * TQ:(tqt + 1) * TQ, h * DH:(h + 1) * DH]
                    nc.sync.dma_start(
                        ydst.rearrange("(j p) c -> p j c", p=P), ysb[:]
                    )

                def attention(h, tqt):
                    g = h >> 1
                    rb = DH * (h & 1)
                    npair = 2 * (tqt + 1)
                    tq0 = tqt * TQ
                    pso = ps_o.tile([VP, TQ], F32, name="pso", tag="o")

                    def emit_pv(et, mp, offs, last=False):
                        for u in range(2):
                            nc.tensor.matmul(
                                pso[:, offs[u]:TQ],
                                vO_sb[:, 2 * mp + u, h, 0:VP],
                                et[:, u, offs[u]:TQ],
                                start=(mp == 0 and u == 0),
                                stop=(last and u == 1),
                                skip_group_check=True,
                            )

                    prev = None
                    for mp in range(npair):
                        # per-block valid-column offsets within the tq tile
                        # (0 off-diagonal; 128*m - tq0 on the diagonal band)
                        offs = [max(0, P * (2 * mp + u) - tq0) for u in (0, 1)]
                        diag = offs[1] > 0
                        pssc = ps_s.tile([P, 2, TQ], F32, name="pssc", tag="s")
                        for u in range(2):
                            m = 2 * mp + u
                            nc.tensor.matmul(
                                pssc[:, u, offs[u]:TQ],
                                kT_sb[rb:rb + DH, g, m, :],
                                qT_sb[rb:rb + DH, g,
                                      tq0 + offs[u]:tq0 + TQ],
                                start=True,
                                stop=True,
                            )
                        et = wpool.tile([P, 2, TQ], FP8, name="et", tag="et")
                        # exp is pair-granular (one instr over both u); on the
                        # diagonal it covers u=1 columns QK didn't write --
                        # stale-but-finite PSUM (zero-initialized at boot),
                        # and the fused dm mask zeroes exactly that region
                        # plus the two block triangles.
                        nc.scalar.activation(
                            et[:, :, offs[0]:TQ], pssc[:, :, offs[0]:TQ],
                            mybir.ActivationFunctionType.Exp,
                            scale=float(SCALE),
                        )
                        if diag:
                            nc.vector.tensor_mul(
                                et[:, :, offs[0]:offs[0] + 2 * P],
                                et[:, :, offs[0]:offs[0] + 2 * P],
                                dm_sb[:],
                            )
                        if prev is not None:
                            emit_pv(*prev)
                        prev = (et, mp, offs)
                        if pending and mp >= 1:
                            epilogue(pending.popleft())
                    emit_pv(*prev, last=True)
                    pending.append(epilogue_start(h, tqt, pso))

                def qproj(tqc):
                    sl = bass.ts(tqc, TQ)
                    for g in range(EG):
                        psq = ps_s.tile([P, TQ], F32, name="psq", tag="s")
                        for dt_i in range(DT):
                            nc.tensor.matmul(
                                psq[:],
                                wq_sb[:, dt_i, P * g:P * (g + 1)],
                                xT_sb[:, dt_i, sl],
                                start=(dt_i == 0),
                                stop=(dt_i == DT - 1),
                            )
                        nc.vector.tensor_copy(qT_sb[:, g, sl], psq[:])

                # ---- main schedule: qproj for tile t, then attention over
                # its heads while stage t+1 streams in.
                qproj(0)
                for tqt in range(NTQ):
                    if tqt + 1 < NTQ:
                        load_stage(tqt + 1)
                    for h in range(HPC):
                        attention(h, tqt)
                        if h == 0 and tqt + 1 < NTQ:
                            qproj(tqt + 1)
                while pending:
                    epilogue(pending.popleft())

            if repeat == 1:
                body()
            else:
                tc.For_i_unrolled(0, repeat, 1, body, max_unroll=1)

    nc.compile()
    return nc


def _f8(a: np.ndarray) -> np.ndarray:
    return np.ascontiguousarray(a, dtype=np.float32).astype(
        ml_dtypes.float8_e4m3
    )


def _b16(a: np.ndarray) -> np.ndarray:
    return np.ascontiguousarray(a, dtype=np.float32).astype(ml_dtypes.bfloat16)


def prep_in_maps(x, k, v, Wq, variant: str = VARIANT):
    """Build the 8 per-core input maps from full inputs (host-side numpy)."""
    x = np.asarray(x, dtype=np.float32)
    k = np.asarray(k, dtype=np.float32)
    v = np.asarray(v, dtype=np.float32)
    Wq = np.asarray(Wq, dtype=np.float32)

    # dm8: fused diagonal mask for a key-block pair at its diagonal corner.
    i_ = np.arange(P)[:, None]
    j_ = np.arange(P)[None, :]
    tri = (i_ <= j_).astype(np.float32)
    dm = np.empty((P, 2, 2 * P), dtype=np.float32)
    dm[:, 0, 0:P] = tri
    dm[:, 0, P:] = 1.0
    dm[:, 1, 0:P] = 0.0
    dm[:, 1, P:] = tri
    ident = np.eye(P, dtype=np.float32)

    in_maps = []
    for c in range(N_CORES):
        b = c // (N_CORES // B)
        grp = c % (N_CORES // B)
        heads = slice(HPC * grp, HPC * (grp + 1))
        cols = slice(EPC * grp, EPC * (grp + 1))

        # xT8[p, dt, t] = x[b, t, 128*dt + p]
        xT8 = x[b].T.reshape(DT, P, T).transpose(1, 0, 2)
        # wq8[p, dt, f] = Wq[cols][f, 128*dt + p]
        wq8 = Wq[cols, :].T.reshape(DT, P, EPC).transpose(1, 0, 2)
        # kT8[64*(h&1)+dh, h>>1, m, tk] = k[b, head h, 128m+tk, dh]
        kT8 = np.zeros((P, EG, NTKB, P), dtype=np.float32)
        for hl in range(HPC):
            g, hb = hl // 2, hl % 2
            kh = k[b, HPC * grp + hl].reshape(NTKB, P, DH)
            kT8[DH * hb:DH * hb + DH, g, :, :] = kh.transpose(2, 0, 1)
        # vO8[p, m, h, dh'] = v[b, head h, 128m+p, dh'], ones at dh'=64
        vO8 = np.ones((P, NTKB, HPC, VP), dtype=np.float32)
        vh = v[b, heads].reshape(HPC, NTKB, P, DH)
        vO8[:, :, :, 0:DH] = vh.transpose(2, 1, 0, 3)
        xres = np.ascontiguousarray(
            x[b][:, cols].reshape(NTKB, P, EPC).transpose(1, 0, 2)
        )
        in_maps.append({
            "xT8": _f8(xT8),
            "wq8": _f8(wq8),
            "kT8": _f8(kT8),
            "vO8": _f8(vO8),
            "xr": _b16(xres),
            "dm8": _f8(dm),
            "ident": _b16(ident),
        })
    return in_maps


def gather_output(results):
    """Assemble full [B, T, D] output from 8 per-core [T, EPC] slices."""
    y = np.empty((B, T, D), dtype=np.float32)
    for c in range(N_CORES):
        b = c // (N_CORES // B)
        grp = c % (N_CORES // B)
        y[b, :, EPC * grp:EPC * (grp + 1)] = np.asarray(
            results[c]["y"], dtype=np.float32
        )
    return y


_NC_CACHE = {}


def kernel(x, k, v, Wq):
    key = (VARIANT, 1)
    if key not in _NC_CACHE:
        _NC_CACHE[key] = build_nc(VARIANT, repeat=1)
    nc = _NC_CACHE[key]
    in_maps = prep_in_maps(x, k, v, Wq, VARIANT)
    res = run_bass_kernel_spmd(nc, in_maps, core_ids=list(range(N_CORES)))
    return gather_output(res.results)
